# revision 12
# baseline (speedup 1.0000x reference)
"""MoE routing kernel for Trainium2 (Bass/Tile), data-parallel over batch on 8 cores.

Reference computation (per token):
  h  = LayerNorm(x) * gamma + beta
  g  = gelu(h @ gw1 + gb1); gate = softmax(g @ gw2 + gb2)
  top2 of gate, weights renormalized
  hid = gelu(h @ ew1[e] + eb1[e]);  eo[e] = hid @ ew2[e] + eb2[e]   (all experts)
  result = sum_k v_k * eo[topk_k]
Outputs: (result [B,D], gate [B,E], eo [B,E,D])

Sharding: batch B=8192 split 8 ways (1024 tokens/core); weights replicated.
Expert matmuls run as float32r (PE rounds operands to 11 mantissa bits, full
rate at N>=256, fp32 storage).  The gating path (which decides top-2 expert
selection) keeps an exact fp32 copy of h (hT) and of g (gTf), so selection
error stays ~1e-6, far below the minimum logit gap; experts use a separately
stored pre-rounded copy hTr since the verifier requires fp32r matmul inputs
to be rounded at store.
"""
import numpy as np

import concourse.bacc as bacc
import concourse.mybir as mybir
from concourse import bass, masks
from concourse.tile import TileContext
from concourse.bass_utils import run_bass_kernel_spmd

F32 = mybir.dt.float32
F32R = mybir.dt.float32r
AF = mybir.ActivationFunctionType
OP = mybir.AluOpType
AX = mybir.AxisListType

B, D, Gh, E, H = 8192, 768, 384, 8, 3072
NCORES = 8
Bc = B // NCORES          # tokens per core
T = Bc // 128             # 8 token tiles of 128
DC = D // 128             # 6 contraction chunks over D
HC = H // 128             # 24 chunks over H
GS = Gh // 128            # 3 slices over Gh
DS = 2                    # D output split (2 x 384)
DSW = D // DS             # 384
LN_EPS = 1e-5


def r(ap):
    """View an fp32 AP as float32r."""
    return ap.bitcast(F32R)


def build_kernel():
    nc = bacc.Bacc("TRN2", target_bir_lowering=False, debug=False)

    x = nc.dram_tensor("x", [Bc, D], F32, kind="ExternalInput")
    ln_gamma = nc.dram_tensor("ln_gamma", [D], F32, kind="ExternalInput")
    ln_beta = nc.dram_tensor("ln_beta", [D], F32, kind="ExternalInput")
    gw1 = nc.dram_tensor("gw1", [D, Gh], F32, kind="ExternalInput")
    gb1 = nc.dram_tensor("gb1", [Gh], F32, kind="ExternalInput")
    gw2 = nc.dram_tensor("gw2", [Gh, E], F32, kind="ExternalInput")
    gb2 = nc.dram_tensor("gb2", [E], F32, kind="ExternalInput")
    ew1 = nc.dram_tensor("ew1", [E, D, H], F32, kind="ExternalInput")
    eb1 = nc.dram_tensor("eb1", [E, H], F32, kind="ExternalInput")
    ew2 = nc.dram_tensor("ew2", [E, H, D], F32, kind="ExternalInput")
    eb2 = nc.dram_tensor("eb2", [E, D], F32, kind="ExternalInput")

    res_out = nc.dram_tensor("res", [Bc, D], F32, kind="ExternalOutput")
    gate_out = nc.dram_tensor("gate", [Bc, E], F32, kind="ExternalOutput")
    eo_out = nc.dram_tensor("eo", [Bc, E, D], F32, kind="ExternalOutput")

    with TileContext(nc) as tc:
        with (
            tc.tile_pool(name="consts", bufs=1) as consts,
            tc.tile_pool(name="persist", bufs=1) as persist,
            tc.tile_pool(name="xp", bufs=2) as xp,
            tc.tile_pool(name="xn", bufs=2) as xnp,
            tc.tile_pool(name="eb2p", bufs=2) as eb2p,
            tc.tile_pool(name="ln", bufs=8) as lnp,
            tc.tile_pool(name="sm", bufs=4) as smp,
            tc.tile_pool(name="ew1p", bufs=4) as ew1p,
            tc.tile_pool(name="ew2p", bufs=4) as ew2p,
            tc.tile_pool(name="eosb", bufs=5) as eosb,
            tc.tile_pool(name="psA", bufs=2, space="PSUM") as psA,   # [128,512] mm acc
            tc.tile_pool(name="psB", bufs=4, space="PSUM") as psB,   # [128,384] eo acc
            tc.tile_pool(name="psC", bufs=2, space="PSUM") as psC,   # transpose + z
        ):
            # ---- constants / small weights ----
            identity = consts.tile([128, 128], F32)
            masks.make_identity(nc, identity[:])
            ones_f = consts.tile([1, 128], F32)
            nc.vector.memset(ones_f[:], 1.0)
            ones_r = consts.tile([1, 128], F32R)
            nc.scalar.activation(ones_r[:], ones_f[:], AF.Identity,
                                 bias=0.0, scale=1.0)

            gamma_sb = consts.tile([128, DC], F32)
            beta_sb = consts.tile([128, DC], F32)
            nc.sync.dma_start(gamma_sb[:], ln_gamma.rearrange("(c p) -> p c", p=128))
            nc.sync.dma_start(beta_sb[:], ln_beta.rearrange("(c p) -> p c", p=128))

            gw1_sb = consts.tile([128, DC, Gh], F32)
            for dc in range(DC):
                nc.sync.dma_start(gw1_sb[:, dc, :], gw1[dc * 128:(dc + 1) * 128, :])
            gb1_sb = consts.tile([128, GS], F32)
            nc.sync.dma_start(gb1_sb[:], gb1.rearrange("(c p) -> p c", p=128))
            gw2_sb = consts.tile([128, GS, E], F32)
            for gs in range(GS):
                nc.sync.dma_start(gw2_sb[:, gs, :], gw2[gs * 128:(gs + 1) * 128, :])
            gb2_row = consts.tile([1, E], F32)
            nc.sync.dma_start(gb2_row[:], gb2.rearrange("(a e) -> a e", a=1))

            eb1_sb = consts.tile([128, E, HC], F32)
            for e in range(E):
                nc.sync.dma_start(
                    eb1_sb[:, e, :], eb1[e].rearrange("(c p) -> p c", p=128)
                )

            # ---- persistent activations ----
            hT = [persist.tile([128, Bc], F32, tag=f"hT{dc}", name=f"hT{dc}")
                  for dc in range(DC)]           # exact fp32 h (gating)
            hTr = [persist.tile([128, Bc], F32R, tag=f"hTr{dc}", name=f"hTr{dc}")
                   for dc in range(DC)]          # rounded h (experts)
            gTf = [persist.tile([128, Bc], F32, tag=f"gTf{gs}", name=f"gTf{gs}")
                   for gs in range(GS)]          # exact fp32 g (gating)
            hid = [persist.tile([128, 512], F32R, tag=f"hid{hc}", name=f"hid{hc}")
                   for hc in range(HC)]          # expert hidden, one token half
            acc = [persist.tile([128, D], F32, tag=f"acc{t}", name=f"acc{t}")
                   for t in range(T)]
            w_all = persist.tile([128, T * E], F32)

            # ---- phase A: LayerNorm + transpose into hT / hTr ----
            for t in range(T):
                x_t = xp.tile([128, D], F32)
                nc.sync.dma_start(x_t[:], x[t * 128:(t + 1) * 128, :])

                sums = lnp.tile([128, 1], F32, tag="lns")
                nc.vector.reduce_sum(sums[:], x_t[:], axis=AX.X)
                negmean = lnp.tile([128, 1], F32, tag="lns")
                nc.vector.tensor_scalar_mul(negmean[:], sums[:], -1.0 / D)
                xc = xnp.tile([128, D], F32, tag="xc")
                nc.vector.tensor_scalar_add(xc[:], x_t[:], negmean[:])

                ssq = lnp.tile([128, 1], F32, tag="lns")
                nc.vector.scalar_tensor_tensor(
                    x_t[:], xc[:], 1.0, xc[:], op0=OP.mult, op1=OP.mult,
                    accum_out=ssq[:],
                )
                veps = lnp.tile([128, 1], F32, tag="lns")
                nc.vector.tensor_scalar(
                    veps[:], ssq[:], 1.0 / D, LN_EPS, op0=OP.mult, op1=OP.add
                )
                sig = lnp.tile([128, 1], F32, tag="lns")
                nc.scalar.sqrt(sig[:], veps[:])
                rsig = lnp.tile([128, 1], F32, tag="lns")
                nc.vector.reciprocal(rsig[:], sig[:])

                nc.vector.tensor_scalar_mul(xc[:], xc[:], rsig[:])

                for dc in range(DC):
                    tp = psC.tile([128, 128], F32, tag="ps_small")
                    nc.tensor.transpose(
                        tp[:], xc[:, dc * 128:(dc + 1) * 128], identity[:]
                    )
                    nc.scalar.activation(
                        hT[dc][:, t * 128:(t + 1) * 128], tp[:], AF.Identity,
                        bias=beta_sb[:, dc:dc + 1], scale=gamma_sb[:, dc:dc + 1],
                    )
                    nc.scalar.activation(
                        hTr[dc][:, t * 128:(t + 1) * 128], tp[:], AF.Identity,
                        bias=beta_sb[:, dc:dc + 1], scale=gamma_sb[:, dc:dc + 1],
                    )

            # ---- phase B: gating MLP (exact fp32) ----
            for gs in range(GS):
                for th in range(2):
                    gp = psA.tile([128, 512], F32, tag="ps_big")
                    for dc in range(DC):
                        nc.tensor.matmul(
                            gp[:],
                            gw1_sb[:, dc, gs * 128:(gs + 1) * 128],
                            hT[dc][:, th * 512:(th + 1) * 512],
                            start=(dc == 0), stop=(dc == DC - 1),
                        )
                    nc.scalar.activation(
                        gTf[gs][:, th * 512:(th + 1) * 512], gp[:], AF.Gelu,
                        bias=gb1_sb[:, gs:gs + 1], scale=1.0,
                    )

            for t in range(T):
                z = psC.tile([128, E], F32, tag="ps_small")
                for gs in range(GS):
                    nc.tensor.matmul(
                        z[:],
                        gTf[gs][:, t * 128:(t + 1) * 128],
                        gw2_sb[:, gs, :],
                        start=(gs == 0), stop=False,
                    )
                nc.tensor.matmul(
                    z[:], ones_f[:], gb2_row[:], start=False, stop=True
                )

                zmax = smp.tile([128, 1], F32, tag="sm1")
                nc.vector.reduce_max(zmax[:], z[:], axis=AX.X)
                negzmax = smp.tile([128, 1], F32, tag="sm1")
                nc.vector.tensor_scalar_mul(negzmax[:], zmax[:], -1.0)
                esum = smp.tile([128, 1], F32, tag="sm1")
                gate_sb = smp.tile([128, E], F32, tag="sm8")
                nc.scalar.activation(
                    gate_sb[:], z[:], AF.Exp, bias=negzmax[:], scale=1.0,
                    accum_out=esum[:],
                )
                rsum = smp.tile([128, 1], F32, tag="sm1")
                nc.vector.reciprocal(rsum[:], esum[:])
                nc.vector.tensor_scalar_mul(gate_sb[:], gate_sb[:], rsum[:])
                nc.sync.dma_start(gate_out[t * 128:(t + 1) * 128, :], gate_sb[:])

                # top-2 selection on the exact fp32 logits in PSUM
                mask1 = smp.tile([128, E], F32, tag="sm8")
                nc.vector.tensor_scalar(
                    mask1[:], z[:], zmax[:], None, op0=OP.is_ge
                )
                maskedz = smp.tile([128, E], F32, tag="sm8")
                nc.vector.scalar_tensor_tensor(
                    maskedz[:], mask1[:], -1e30, z[:], op0=OP.mult, op1=OP.add
                )
                m2z = smp.tile([128, 1], F32, tag="sm1")
                nc.vector.reduce_max(m2z[:], maskedz[:], axis=AX.X)
                mask2 = smp.tile([128, E], F32, tag="sm8")
                nc.vector.tensor_scalar(
                    mask2[:], maskedz[:], m2z[:], None, op0=OP.is_ge
                )
                # gate values at the two selected positions (row sums)
                tmp8 = smp.tile([128, E], F32, tag="sm8")
                v1 = smp.tile([128, 1], F32, tag="sm1")
                nc.vector.scalar_tensor_tensor(
                    tmp8[:], mask1[:], 1.0, gate_sb[:], op0=OP.mult, op1=OP.mult,
                    accum_out=v1[:],
                )
                v2 = smp.tile([128, 1], F32, tag="sm1")
                nc.vector.scalar_tensor_tensor(
                    tmp8[:], mask2[:], 1.0, gate_sb[:], op0=OP.mult, op1=OP.mult,
                    accum_out=v2[:],
                )
                denom = smp.tile([128, 1], F32, tag="sm1")
                nc.vector.tensor_tensor(denom[:], v1[:], v2[:], op=OP.add)
                rden = smp.tile([128, 1], F32, tag="sm1")
                nc.vector.reciprocal(rden[:], denom[:])
                v1n = smp.tile([128, 1], F32, tag="sm1")
                nc.vector.tensor_tensor(v1n[:], v1[:], rden[:], op=OP.mult)
                v2n = smp.tile([128, 1], F32, tag="sm1")
                nc.vector.tensor_tensor(v2n[:], v2[:], rden[:], op=OP.mult)
                nc.vector.tensor_scalar(
                    tmp8[:], mask1[:], v1n[:], None, op0=OP.mult
                )
                nc.vector.scalar_tensor_tensor(
                    w_all[:, t * E:(t + 1) * E], mask2[:], v2n[:], tmp8[:],
                    op0=OP.mult, op1=OP.add,
                )

            # ---- phase C: experts (token halves of 512) ----
            for e in range(E):
                eb2_e = eb2p.tile([1, D], F32R, tag="eb2e", name=f"eb2e{e}")
                nc.sync.dma_start(eb2_e[:], r(eb2[e].rearrange("(a f) -> a f", a=1)))
                for th in range(2):
                    # stage 1: hid = gelu(h @ ew1[e] + eb1[e]), transposed layout
                    for hs in range(HC):
                        w1t = ew1p.tile([128, DC, 128], F32R, tag="ew1",
                                        name=f"w1t{e}_{th}_{hs}")
                        nc.sync.dma_start(
                            w1t[:],
                            r(ew1[e].rearrange("(c p) h -> p c h", p=128)
                              [:, :, hs * 128:(hs + 1) * 128]),
                        )
                        hp = psA.tile([128, 512], F32, tag="ps_big")
                        for dc in range(DC):
                            nc.tensor.matmul(
                                hp[:],
                                w1t[:, dc, :],
                                hTr[dc][:, th * 512:(th + 1) * 512],
                                start=(dc == 0), stop=(dc == DC - 1),
                            )
                        nc.scalar.activation(
                            hid[hs][:], hp[:], AF.Gelu,
                            bias=eb1_sb[:, e, hs:hs + 1], scale=1.0,
                        )

                    # stage 2: eo = hid @ ew2[e] + eb2[e]; combine top-2 into acc
                    eo_full = [eosb.tile([128, D], F32, tag="eo_sb",
                                         name=f"eof{e}_{th}_{i}") for i in range(4)]
                    for ds in range(DS):
                        pe_t = [psB.tile([128, DSW], F32, tag="ps_eo",
                                         name=f"pe{e}_{th}_{ds}_{i}")
                                for i in range(4)]
                        for hc in range(HC):
                            w2t = ew2p.tile([128, DSW], F32R, tag="ew2",
                                            name=f"w2t{e}_{th}_{ds}_{hc}")
                            nc.sync.dma_start(
                                w2t[:],
                                r(ew2[e, hc * 128:(hc + 1) * 128,
                                      ds * DSW:(ds + 1) * DSW]),
                            )
                            for tl in range(4):
                                nc.tensor.matmul(
                                    pe_t[tl][:],
                                    hid[hc][:, tl * 128:(tl + 1) * 128],
                                    w2t[:],
                                    start=(hc == 0), stop=False,
                                )
                        for tl in range(4):
                            t = th * 4 + tl
                            nc.tensor.matmul(
                                pe_t[tl][:], ones_r[:],
                                eb2_e[:, ds * DSW:(ds + 1) * DSW],
                                start=False, stop=True,
                            )
                            eo_t = eo_full[tl][:, ds * DSW:(ds + 1) * DSW]
                            nc.vector.tensor_copy(eo_t, pe_t[tl][:])
                            wcol = w_all[:, t * E + e: t * E + e + 1]
                            if e == 0:
                                nc.vector.tensor_scalar(
                                    acc[t][:, ds * DSW:(ds + 1) * DSW],
                                    eo_t, wcol, None, op0=OP.mult,
                                )
                            else:
                                nc.vector.scalar_tensor_tensor(
                                    acc[t][:, ds * DSW:(ds + 1) * DSW],
                                    eo_t, wcol,
                                    acc[t][:, ds * DSW:(ds + 1) * DSW],
                                    op0=OP.mult, op1=OP.add,
                                )
                            if ds == DS - 1:
                                nc.sync.dma_start(
                                    eo_out[t * 128:(t + 1) * 128, e, :],
                                    eo_full[tl][:],
                                )

            for t in range(T):
                nc.sync.dma_start(res_out[t * 128:(t + 1) * 128, :], acc[t][:])

    nc.compile()
    return nc


_NC_CACHE = None


def kernel(pooler_out, ln_gamma, ln_beta, gw1, gb1, gw2, gb2, ew1, eb1, ew2, eb2,
           top_k=2, **_unused):
    global _NC_CACHE
    assert int(top_k) == 2
    if _NC_CACHE is None:
        _NC_CACHE = build_kernel()
    nc = _NC_CACHE

    f = lambda a: np.ascontiguousarray(np.asarray(a), dtype=np.float32)
    shared = {
        "ln_gamma": f(ln_gamma), "ln_beta": f(ln_beta),
        "gw1": f(gw1), "gb1": f(gb1), "gw2": f(gw2), "gb2": f(gb2),
        "ew1": f(ew1), "eb1": f(eb1), "ew2": f(ew2), "eb2": f(eb2),
    }
    xfull = f(pooler_out)
    in_maps = [
        {"x": xfull[c * Bc:(c + 1) * Bc], **shared} for c in range(NCORES)
    ]
    res = run_bass_kernel_spmd(nc, in_maps, core_ids=list(range(NCORES)))
    result = np.concatenate([res.results[c]["res"] for c in range(NCORES)], axis=0)
    gate = np.concatenate([res.results[c]["gate"] for c in range(NCORES)], axis=0)
    eo = np.concatenate([res.results[c]["eo"] for c in range(NCORES)], axis=0)
    return result, gate, eo
